# revision 8
# baseline (speedup 1.0000x reference)
"""Trainium2 Bass kernel for nn_BigNet: 1000x (Linear(100,100)+ReLU) -> Linear(100,10).

fp16 4-chunk design (data-parallel, batch 8192 -> 1024 cols/core):
  - All matmul inputs fp16 (end-to-end max rel err ~5e-3 vs the 2e-2 gate);
    PSUM accumulates fp32. fp16 streams 2 bytes/col so the PE matmul track
    (the baseline's 96%-busy bottleneck at f32r) drops well under the
    ReLU-eviction cost.
  - 4 column chunks (ACT,DVE,ACT,DVE) staggered as in the baseline: with 4
    independent chunk chains in flight, each matmul's fused LDWEIGHTS
    prefetches while the previous chunk streams, keeping it off the
    mm->relu->mm critical chain (a 2-chunk variant measured 1412ns/layer
    because LDW serialized behind the relu semaphore).
  - Chunk widths 227/285 balance ACT (FD+315cyc)/1.2GHz vs DVE
    (FD+152cyc)/0.96GHz per-op busy (~450ns each).
  - GPSIMD cannot access PSUM (and is ~14.6ns/col even SBUF->SBUF), so
    eviction is ACT+DVE only. Weight DMAs are descriptor-rate-bound (~101
    partition lines x ~80ns), so groups are 20 layers (4000B lines) and
    alternate between the sync and gpsimd queues; smaller fp16 groups of 8
    starved the pipeline (~8.3us stall at every group boundary).
"""

import sys

if "/opt/trn_rl_repo" not in sys.path:
    sys.path.insert(0, "/opt/trn_rl_repo")

import numpy as np

N_LAYERS, D, D_OUT, B, N_CORES = 1000, 100, 10, 8192, 8
K = D + 1  # augmented contraction dim (ones row carries the bias)
B_CORE = B // N_CORES

# chunk widths and relu engine per chunk (A=ScalarE ACT, V=VectorE DVE)
CHUNKS = (227, 227, 285, 285)
ENGS = ("A", "A", "V", "V")
W_PER_DMA = 20

_BUILT = {}


def _build():
    import concourse.bacc as bacc
    import concourse.mybir as mybir
    from concourse.tile import TileContext

    f16 = mybir.dt.float16
    f32 = mybir.dt.float32

    offs = [0]
    for cw in CHUNKS:
        offs.append(offs[-1] + cw)
    assert offs[-1] == B_CORE

    nc = bacc.Bacc(None, target_bir_lowering=False)
    n_groups = N_LAYERS // W_PER_DMA
    # Weights ship as f32 pairs (DMA rate is per-element: 2-byte fp16
    # elements halve bandwidth) and are bitcast to fp16 in SBUF.
    wt_e = nc.dram_tensor(
        "wt", [n_groups, K, D * W_PER_DMA // 2], f32, kind="ExternalInput"
    )
    # xt packed per chunk: [K*offs[c], K*offs[c+1]) as [K, cw] row-major
    xt_e = nc.dram_tensor("xt", [K * B_CORE], f16, kind="ExternalInput")
    wft_e = nc.dram_tensor("wft", [K, D_OUT], f16, kind="ExternalInput")
    out_e = nc.dram_tensor("out", [D_OUT, B_CORE], f32, kind="ExternalOutput")

    with TileContext(nc) as tc:
        with (
            tc.tile_pool(name="h", bufs=1) as hpool,
            tc.tile_pool(name="w", bufs=4) as wpool,
            tc.tile_pool(name="ps", bufs=1, space="PSUM") as pspool,
            tc.tile_pool(name="misc", bufs=1) as mpool,
        ):
            # Prefetch the first two weight groups before activations.
            wtiles0 = []
            for g0 in range(2):
                wt_t = wpool.tile(
                    [K, D * W_PER_DMA // 2], f32, tag="w", name="wtile"
                )
                eng = nc.sync if g0 % 2 == 0 else nc.gpsimd
                eng.dma_start(wt_t[:], wt_e[g0])
                wtiles0.append(wt_t)

            wf_tile = mpool.tile([K, D_OUT], f16, tag="wf")
            nc.sync.dma_start(wf_tile[:], wft_e[:])

            # Ping-pong activation tiles per chunk; row D holds the 1.0 that
            # multiplies the bias row of the augmented weights.
            h = [
                [
                    hpool.tile([K, CHUNKS[c]], f16, tag=f"h{p}_{c}", name=f"h{p}_{c}")
                    for c in range(len(CHUNKS))
                ]
                for p in range(2)
            ]
            for c, cw in enumerate(CHUNKS):
                blk = xt_e[K * offs[c] : K * offs[c + 1]].rearrange(
                    "(k w) -> k w", w=cw
                )
                nc.gpsimd.dma_start(h[0][c][:], blk)
                nc.gpsimd.dma_start(h[1][c][D:K, :], blk[D:K, :])  # ones row

            wtile = None
            for l in range(N_LAYERS):
                j = l % W_PER_DMA
                if j == 0:
                    g = l // W_PER_DMA
                    if g < 2:
                        wtile = wtiles0[g]
                    else:
                        wtile = wpool.tile(
                            [K, D * W_PER_DMA // 2], f32, tag="w", name="wtile"
                        )
                        eng = nc.sync if g % 2 == 0 else nc.gpsimd
                        eng.dma_start(wtile[:], wt_e[g])
                wsl = wtile[:].bitcast(f16)[:, j * D : (j + 1) * D]
                p, q = l % 2, (l + 1) % 2
                for c, cw in enumerate(CHUNKS):
                    ps = pspool.tile([D, cw], f32, tag=f"ps{c}", name=f"ps{c}")
                    nc.tensor.matmul(ps[:], wsl, h[p][c][:], start=True, stop=True)
                    if ENGS[c] == "A":
                        nc.scalar.activation(
                            h[q][c][0:D, :],
                            ps[:],
                            mybir.ActivationFunctionType.Relu,
                        )
                    else:
                        nc.vector.tensor_scalar_max(h[q][c][0:D, :], ps[:], 0.0)

            # Final Linear(100 -> 10), no ReLU.
            out_sb = mpool.tile([D_OUT, B_CORE], f32, tag="out")
            pf = N_LAYERS % 2
            for c, cw in enumerate(CHUNKS):
                ps = pspool.tile([D_OUT, cw], f32, tag=f"ps{c}", name=f"psf{c}")
                nc.tensor.matmul(ps[:], wf_tile[:], h[pf][c][:], start=True, stop=True)
                nc.scalar.copy(out_sb[:, offs[c] : offs[c + 1]], ps[:])
            nc.sync.dma_start(out_e[:], out_sb[:])

    nc.finalize()
    return nc


def _get_nc():
    nc = _BUILT.get("v4")
    if nc is None:
        nc = _build()
        _BUILT["v4"] = nc
    return nc


def _prep_inputs(x, W, b, Wf, bf):
    x = np.asarray(x, dtype=np.float32)
    W = np.asarray(W, dtype=np.float32)
    b = np.asarray(b, dtype=np.float32)
    Wf = np.asarray(Wf, dtype=np.float32)
    bf = np.asarray(bf, dtype=np.float32)

    # wt[g, p, j*D + m] = Waug[g*W_PER_DMA + j, p, m], Waug[l] = [W[l].T ; b[l]]
    waug = np.concatenate([W.transpose(0, 2, 1), b[:, None, :]], axis=1)
    n_groups = N_LAYERS // W_PER_DMA
    wt = np.ascontiguousarray(
        waug.reshape(n_groups, W_PER_DMA, K, D)
        .transpose(0, 2, 1, 3)
        .reshape(n_groups, K, W_PER_DMA * D)
    ).astype(np.float16)
    wt = wt.view(np.float32)  # DMA as 4-byte elements for full rate

    xt = np.empty((K, B), dtype=np.float16)
    xt[:D] = x.T.astype(np.float16)
    xt[D] = 1.0
    offs = [0]
    for cw in CHUNKS:
        offs.append(offs[-1] + cw)
    xt_packed = np.empty((N_CORES, K * B_CORE), dtype=np.float16)
    for i in range(N_CORES):
        col0 = i * B_CORE
        for c, cw in enumerate(CHUNKS):
            xt_packed[i, K * offs[c] : K * offs[c + 1]] = xt[
                :, col0 + offs[c] : col0 + offs[c + 1]
            ].ravel()

    wft = np.ascontiguousarray(
        np.concatenate([Wf.T, bf[None, :]], axis=0)
    ).astype(np.float16)
    return wt, xt_packed, wft


def run(x, W, b, Wf, bf, mm_dtype=None, trace=False):
    from concourse.bass_utils import run_bass_kernel_spmd

    nc = _get_nc()
    wt, xt_packed, wft = _prep_inputs(x, W, b, Wf, bf)
    in_maps = [
        {"wt": wt, "xt": xt_packed[i], "wft": wft} for i in range(N_CORES)
    ]
    res = run_bass_kernel_spmd(
        nc, in_maps, core_ids=list(range(N_CORES)), trace=trace
    )
    out = np.concatenate([res.results[i]["out"] for i in range(N_CORES)], axis=1)
    return np.ascontiguousarray(out.T, dtype=np.float32), res


def kernel(x, W, b, Wf, bf):
    out, _ = run(x, W, b, Wf, bf)
    return out


# revision 9
# speedup vs baseline: 1.3811x; 1.3811x over previous
"""Trainium2 Bass kernel for nn_BigNet: 1000x (Linear(100,100)+ReLU) -> Linear(100,10).

fp16 4-chunk design (data-parallel, batch 8192 -> 1024 cols/core):
  - All matmul inputs fp16 (end-to-end max rel err ~5e-3 vs the 2e-2 gate);
    PSUM accumulates fp32. fp16 streams 2 bytes/col so the PE matmul track
    (the baseline's 96%-busy bottleneck at f32r) drops well under the
    ReLU-eviction cost.
  - 4 column chunks (ACT,DVE,ACT,DVE) staggered as in the baseline: with 4
    independent chunk chains in flight, each matmul's fused LDWEIGHTS
    prefetches while the previous chunk streams, keeping it off the
    mm->relu->mm critical chain (a 2-chunk variant measured 1412ns/layer
    because LDW serialized behind the relu semaphore).
  - Chunk widths 227/285 balance ACT (FD+315cyc)/1.2GHz vs DVE
    (FD+152cyc)/0.96GHz per-op busy (~450ns each).
  - GPSIMD cannot access PSUM (and is ~14.6ns/col even SBUF->SBUF), so
    eviction is ACT+DVE only. Weight DMAs are descriptor-rate-bound (~101
    partition lines x ~80ns), so groups are 20 layers (4000B lines) and
    alternate between the sync and gpsimd queues; smaller fp16 groups of 8
    starved the pipeline (~8.3us stall at every group boundary).
"""

import sys

if "/opt/trn_rl_repo" not in sys.path:
    sys.path.insert(0, "/opt/trn_rl_repo")

import numpy as np

N_LAYERS, D, D_OUT, B, N_CORES = 1000, 100, 10, 8192, 8
K = D + 1  # augmented contraction dim (ones row carries the bias)
B_CORE = B // N_CORES

# chunk widths and relu engine per chunk (A=ScalarE ACT, V=VectorE DVE)
CHUNKS = (227, 227, 285, 285)
ENGS = ("A", "A", "V", "V")
W_PER_DMA = 20

_BUILT = {}


def _build():
    import concourse.bacc as bacc
    import concourse.mybir as mybir
    from concourse.tile import TileContext

    f16 = mybir.dt.float16
    f32 = mybir.dt.float32

    offs = [0]
    for cw in CHUNKS:
        offs.append(offs[-1] + cw)
    assert offs[-1] == B_CORE

    nc = bacc.Bacc(None, target_bir_lowering=False)
    n_groups = N_LAYERS // W_PER_DMA
    # Weights ship as f32 pairs (DMA rate is per-element: 2-byte fp16
    # elements halve bandwidth) and are bitcast to fp16 in SBUF.
    wt_e = nc.dram_tensor(
        "wt", [n_groups, K, D * W_PER_DMA // 2], f32, kind="ExternalInput"
    )
    # xt packed per chunk: [K*offs[c], K*offs[c+1]) as [K, cw] row-major
    xt_e = nc.dram_tensor("xt", [K * B_CORE], f16, kind="ExternalInput")
    wft_e = nc.dram_tensor("wft", [K, D_OUT], f16, kind="ExternalInput")
    out_e = nc.dram_tensor("out", [D_OUT, B_CORE], f32, kind="ExternalOutput")

    with TileContext(nc) as tc:
        with (
            tc.tile_pool(name="h", bufs=1) as hpool,
            tc.tile_pool(name="w", bufs=8) as wpool,
            tc.tile_pool(name="ps", bufs=1, space="PSUM") as pspool,
            tc.tile_pool(name="misc", bufs=1) as mpool,
        ):
            # Prefetch the first four weight groups before activations.
            wtiles0 = []
            for g0 in range(4):
                wt_t = wpool.tile(
                    [K, D * W_PER_DMA // 2], f32, tag="w", name="wtile"
                )
                eng = nc.sync if g0 % 2 == 0 else nc.gpsimd
                eng.dma_start(wt_t[:], wt_e[g0])
                wtiles0.append(wt_t)

            wf_tile = mpool.tile([K, D_OUT], f16, tag="wf")
            nc.sync.dma_start(wf_tile[:], wft_e[:])

            # Ping-pong activation tiles per chunk; row D holds the 1.0 that
            # multiplies the bias row of the augmented weights.
            h = [
                [
                    hpool.tile([K, CHUNKS[c]], f16, tag=f"h{p}_{c}", name=f"h{p}_{c}")
                    for c in range(len(CHUNKS))
                ]
                for p in range(2)
            ]
            for c, cw in enumerate(CHUNKS):
                blk = xt_e[K * offs[c] : K * offs[c + 1]].rearrange(
                    "(k w) -> k w", w=cw
                )
                nc.gpsimd.dma_start(h[0][c][:], blk)
                nc.gpsimd.dma_start(h[1][c][D:K, :], blk[D:K, :])  # ones row

            wtile = None
            for l in range(N_LAYERS):
                j = l % W_PER_DMA
                if j == 0:
                    g = l // W_PER_DMA
                    if g < 4:
                        wtile = wtiles0[g]
                    else:
                        wtile = wpool.tile(
                            [K, D * W_PER_DMA // 2], f32, tag="w", name="wtile"
                        )
                        eng = nc.sync if g % 2 == 0 else nc.gpsimd
                        eng.dma_start(wtile[:], wt_e[g])
                wsl = wtile[:].bitcast(f16)[:, j * D : (j + 1) * D]
                p, q = l % 2, (l + 1) % 2
                for c, cw in enumerate(CHUNKS):
                    ps = pspool.tile([D, cw], f32, tag=f"ps{c}", name=f"ps{c}")
                    nc.tensor.matmul(ps[:], wsl, h[p][c][:], start=True, stop=True)
                    if ENGS[c] == "A":
                        nc.scalar.activation(
                            h[q][c][0:D, :],
                            ps[:],
                            mybir.ActivationFunctionType.Relu,
                        )
                    else:
                        nc.vector.tensor_scalar_max(h[q][c][0:D, :], ps[:], 0.0)

            # Final Linear(100 -> 10), no ReLU.
            out_sb = mpool.tile([D_OUT, B_CORE], f32, tag="out")
            pf = N_LAYERS % 2
            for c, cw in enumerate(CHUNKS):
                ps = pspool.tile([D_OUT, cw], f32, tag=f"ps{c}", name=f"psf{c}")
                nc.tensor.matmul(ps[:], wf_tile[:], h[pf][c][:], start=True, stop=True)
                nc.scalar.copy(out_sb[:, offs[c] : offs[c + 1]], ps[:])
            nc.sync.dma_start(out_e[:], out_sb[:])

    nc.finalize()
    return nc


def _get_nc():
    nc = _BUILT.get("v4")
    if nc is None:
        nc = _build()
        _BUILT["v4"] = nc
    return nc


def _prep_inputs(x, W, b, Wf, bf):
    x = np.asarray(x, dtype=np.float32)
    W = np.asarray(W, dtype=np.float32)
    b = np.asarray(b, dtype=np.float32)
    Wf = np.asarray(Wf, dtype=np.float32)
    bf = np.asarray(bf, dtype=np.float32)

    # wt[g, p, j*D + m] = Waug[g*W_PER_DMA + j, p, m], Waug[l] = [W[l].T ; b[l]]
    waug = np.concatenate([W.transpose(0, 2, 1), b[:, None, :]], axis=1)
    n_groups = N_LAYERS // W_PER_DMA
    wt = np.ascontiguousarray(
        waug.reshape(n_groups, W_PER_DMA, K, D)
        .transpose(0, 2, 1, 3)
        .reshape(n_groups, K, W_PER_DMA * D)
    ).astype(np.float16)
    wt = wt.view(np.float32)  # DMA as 4-byte elements for full rate

    xt = np.empty((K, B), dtype=np.float16)
    xt[:D] = x.T.astype(np.float16)
    xt[D] = 1.0
    offs = [0]
    for cw in CHUNKS:
        offs.append(offs[-1] + cw)
    xt_packed = np.empty((N_CORES, K * B_CORE), dtype=np.float16)
    for i in range(N_CORES):
        col0 = i * B_CORE
        for c, cw in enumerate(CHUNKS):
            xt_packed[i, K * offs[c] : K * offs[c + 1]] = xt[
                :, col0 + offs[c] : col0 + offs[c + 1]
            ].ravel()

    wft = np.ascontiguousarray(
        np.concatenate([Wf.T, bf[None, :]], axis=0)
    ).astype(np.float16)
    return wt, xt_packed, wft


def run(x, W, b, Wf, bf, mm_dtype=None, trace=False):
    from concourse.bass_utils import run_bass_kernel_spmd

    nc = _get_nc()
    wt, xt_packed, wft = _prep_inputs(x, W, b, Wf, bf)
    in_maps = [
        {"wt": wt, "xt": xt_packed[i], "wft": wft} for i in range(N_CORES)
    ]
    res = run_bass_kernel_spmd(
        nc, in_maps, core_ids=list(range(N_CORES)), trace=trace
    )
    out = np.concatenate([res.results[i]["out"] for i in range(N_CORES)], axis=1)
    return np.ascontiguousarray(out.T, dtype=np.float32), res


def kernel(x, W, b, Wf, bf):
    out, _ = run(x, W, b, Wf, bf)
    return out
